# revision 37
# baseline (speedup 1.0000x reference)
"""AKGCN Trainium2 kernel — data-parallel over batch across 8 NeuronCores.

Reference computation (per batch b):
    lam_k = 1 + relu(lambdas[k]);  a_k = (2*lam_k-2)/lam_k;  c_k = 2/lam_k
    layer k:  H = Z @ theta_k                       (feature matmul, C=128)
              V = (a_k*I + c_k*adj) row-normalized  (node matmul, N=512)
              Z' = relu(V @ H)          (vectorized over T)
    out = relu(x) + sigmoid(Z_final)

Device strategy (per core, 4 batches):
  - Activations alternate layouts per matmul so NO on-device transposes
    are needed:
      * feature-major FT: [feat 128 partitions, (t, n) = 8192 free]
      * node-major tiles: [node_local 128, (t, chunk, feat)]
    theta-matmul (bf16) consumes FT (activation tiles as stationary lhsT,
    theta moving) and emits node-major tiles; the adj-matmul consumes
    node-major (H tiles as lhsT, V^T moving 512-wide) and emits FT.
  - adj is supplied PRE-TRANSPOSED (A^T) and x PRE-TRANSPOSED to
    feature-major by the host glue below; both cast to bf16 on host.
  - Row normalization on device: rowsum via ones^T @ A^T matmul,
    rinv ~ 1/(a + c*rowsum); V^T = 128*c*rinv[n]*(A^T + (a/c)*I), built
    once per batch for both layers (step-0 broadcast APs cover the k dim).
  - The adj-matmul runs in fp8e4 DoubleRow perf mode (2 fp8 MACs/cell per
    cycle, K=256 per matmul): V^T is prescaled by 128 to stay in fp8
    normal range and layer-2 H by 1/4; the epilogue activation `scale`
    undoes both before relu/sigmoid. Verified rel err ~2e-3 (fro).
  - Output written feature-major bf16; host restores [T,N,C] f32.
  - M1/M2 emission is software-pipelined (lag-3, matched to the 3-slot
    PSUM ring) and PSUM drains are split across DVE/ScalarE so the
    TensorE never stalls (stalls also cool the HAM clock gate, halving
    the matmul clock).
"""
import sys

if '/opt/trn_rl_repo' not in sys.path:
    sys.path.insert(0, '/opt/trn_rl_repo')

import numpy as np
import ml_dtypes


def _bcast_mid(ap, count):
    # insert a step-0 middle dim: [P, F] -> [P, count, F]
    import concourse.bass as bass
    return bass.AP(ap.tensor, ap.offset,
                   [list(ap.ap[0]), [0, count], list(ap.ap[1])])

B, T, N, C, K = 32, 16, 512, 128, 2
NCORES = 8
BL = B // NCORES  # batches per core
TN = T * N        # 8192
NCH = N // 128    # 4 node chunks
CAST_DVE = 1      # of 8 M1-psum drains per layer, how many go to DVE (rest ScalarE)
RELU_DVE = 3      # of 8 M2 relu drains, how many go to DVE

_cache = {}


def _build():
    from contextlib import ExitStack
    import concourse.tile as tile
    import concourse.mybir as mybir
    from concourse import bacc

    dt = mybir.dt
    f32, bf16, i32 = dt.float32, dt.bfloat16, dt.int32
    fp8 = dt.float8e4
    DR = mybir.MatmulPerfMode.DoubleRow
    AF = mybir.ActivationFunctionType
    OP = mybir.AluOpType

    nc = bacc.Bacc(None, target_bir_lowering=False)
    x_ext = nc.declare_dram_parameter("x", [BL, C, TN], bf16, isOutput=False)
    at_ext = nc.declare_dram_parameter("adjt", [BL, N, N], bf16, isOutput=False)
    lam_ext = nc.declare_dram_parameter("lambdas", [1, K], f32, isOutput=False)
    th_ext = nc.declare_dram_parameter("thetas", [K, C, C], bf16, isOutput=False)
    out_ext = nc.declare_dram_parameter("out", [BL, C, TN], bf16, isOutput=True)

    with tile.TileContext(nc) as tc, ExitStack() as ctx:
        const = ctx.enter_context(tc.tile_pool(name="const", bufs=1))
        xpool = ctx.enter_context(tc.tile_pool(name="x", bufs=2))
        atpool = ctx.enter_context(tc.tile_pool(name="at", bufs=2))
        vtpool = ctx.enter_context(tc.tile_pool(name="vt", bufs=2))
        hpool = ctx.enter_context(tc.tile_pool(name="h", bufs=3))
        spool = ctx.enter_context(tc.tile_pool(name="s", bufs=2))
        opool = ctx.enter_context(tc.tile_pool(name="o", bufs=2))
        small = ctx.enter_context(tc.tile_pool(name="small", bufs=3))
        ps = ctx.enter_context(tc.tile_pool(name="ps", bufs=3, space="PSUM"))
        psr = ctx.enter_context(tc.tile_pool(name="psr", bufs=1, space="PSUM"))

        # ---- batch-0 input DMAs first: the rowsum matmul only needs A^T ----
        preloaded = {}

        def emit_at_dma(b):
            AT = atpool.tile([128, NCH, N], bf16)
            atr = at_ext[b].rearrange("(i p) n -> p i n", p=128)
            for i in range(NCH):
                nc.sync.dma_start(AT[:, i, :], atr[:, i, :])
            return AT

        def emit_x_dma(b):
            X = xpool.tile([C, TN], bf16)
            for h in range(4):
                nc.sync.dma_start(X[:, h * (TN // 4):(h + 1) * (TN // 4)],
                                  x_ext[b][:, h * (TN // 4):(h + 1) * (TN // 4)])
            return X

        def emit_in_dma(b):
            return emit_at_dma(b), emit_x_dma(b)

        AT0 = emit_at_dma(0)

        # ---- one-time constants ----
        th_sb = const.tile([C, K, C], bf16)
        nc.sync.dma_start(th_sb[:], th_ext[:].rearrange("k c d -> c k d"))
        lam_raw = const.tile([1, K], f32)
        nc.sync.dma_start(lam_raw[:], lam_ext[:])
        preloaded[0] = (AT0, emit_x_dma(0))
        ones_sb = const.tile([128, 1], bf16)
        nc.vector.memset(ones_sb[:], 1.0)

        # HAM pre-warm: keep TensorE busy while input DMAs land so the clock
        # gate opens (4/8 -> 8/8) before real matmuls start.
        warm_sb = const.tile([128, 512], bf16)
        nc.vector.memset(warm_sb[:], 0.0)
        wps = psr.tile([1, N], f32, tag="rps")
        for w in range(6):
            nc.tensor.matmul(wps[0:1, 0:N], ones_sb[:, 0:1], warm_sb[:],
                             start=(w == 0), stop=(w == 5))

        # lam = 1 + relu(lam_raw); c = 2/lam; a = 2 - c
        lam_sb = const.tile([1, K], f32)
        nc.vector.tensor_scalar(lam_sb[:], lam_raw[:], 0.0, 1.0, OP.max, OP.add)
        ilam_sb = const.tile([1, K], f32)
        nc.vector.reciprocal(ilam_sb[:], lam_sb[:])
        c_sb = const.tile([1, K], f32)
        nc.vector.tensor_scalar(c_sb[:], ilam_sb[:], 2.0, None, OP.mult)
        a_sb = const.tile([1, K], f32)
        nc.vector.tensor_scalar(a_sb[:], c_sb[:], -1.0, 2.0, OP.mult, OP.add)
        # per-partition copies for scalar_tensor_tensor operands,
        # pre-scaled by 128 (fp8 V^T prescale; undone in the epilogue scale)
        c_bc = const.tile([128, K], f32)
        nc.gpsimd.partition_broadcast(c_bc[:], c_sb[:])
        a_bc = const.tile([128, K], f32)
        nc.gpsimd.partition_broadcast(a_bc[:], a_sb[:])
        ci_bc = const.tile([128, K], f32)
        nc.vector.reciprocal(ci_bc[:], c_bc[:])
        ac_bc = const.tile([128, K], f32)
        nc.vector.tensor_tensor(ac_bc[:], a_bc[:], ci_bc[:], OP.mult)
        cs128_sb = const.tile([1, K], f32)
        nc.vector.tensor_scalar(cs128_sb[:], c_sb[:], 128.0, None, OP.mult)

        # eye128 (f32) via iota(f - p) == 0
        iota_sb = const.tile([128, 128], i32)
        nc.gpsimd.iota(iota_sb[:], pattern=[[1, 128]], base=0, channel_multiplier=-1)
        eye_sb = const.tile([128, 128], f32)
        nc.vector.tensor_scalar(eye_sb[:], iota_sb[:], 0, None, OP.is_equal)
        # eyeac[:, k, :] = (a_k/c_k) * I
        eyeac = const.tile([128, K, 128], f32)
        for k in range(K):
            nc.vector.tensor_scalar(eyeac[:, k, :], eye_sb[:],
                                    ac_bc[:, k:k + 1], None, OP.mult)

        def emit_norm(b):
            if b in preloaded:
                AT, X = preloaded.pop(b)
            else:
                AT, X = emit_in_dma(b)

            # rowsum[n] = sum_m A[n, m] = ones^T @ A^T   -> [1, N] in PSUM
            rps = psr.tile([1, N], f32, tag="rps")
            for i in range(NCH):
                nc.tensor.matmul(rps[0:1, 0:N], ones_sb[:, 0:1], AT[:, i, :],
                                 start=(i == 0), stop=(i == NCH - 1))
            # V^T_k = (128*c_k*rinv_k[n]) * (A^T + (a_k/c_k)*I)   (fp8)
            # RBC[:, k, n] = 128*c_k*rinv_k[n]; both k processed per op via
            # a step-0 broadcast of A^T along the k dim.
            VT = vtpool.tile([128, K, NCH, N], fp8)
            RBC = small.tile([128, K, N], f32)
            for k in range(K):
                rr = small.tile([1, N], f32)
                nc.vector.tensor_scalar(rr[:], rps[0:1, 0:N],
                                        c_sb[0:1, k:k + 1], a_sb[0:1, k:k + 1],
                                        OP.mult, OP.add)
                rinv = small.tile([1, N], f32)
                nc.vector.reciprocal_approx_fast(rinv[:], rr[:])
                rsc = small.tile([1, N], f32)
                nc.vector.tensor_scalar(rsc[:], rinv[:],
                                        cs128_sb[0:1, k:k + 1], None, OP.mult)
                nc.gpsimd.partition_broadcast(RBC[:, k, :], rsc[:])
            for i in range(NCH):
                nc.vector.tensor_tensor(VT[:, :, i, :], RBC[:],
                                        _bcast_mid(AT[:, i, :], K), OP.mult)
                dgs = small.tile([128, K, 128], f32)
                nc.vector.tensor_tensor(dgs[:], eyeac[:],
                                        _bcast_mid(AT[:, i, i * 128:(i + 1) * 128], K),
                                        OP.add)
                nc.vector.tensor_tensor(VT[:, :, i, i * 128:(i + 1) * 128],
                                        RBC[:, :, i * 128:(i + 1) * 128],
                                        dgs[:], OP.mult)
            return AT, VT, X

        for b in range(BL):
            AT, VT, X = emit_norm(b)
            Zin = X
            for k in range(K):
                last = (k == K - 1)
                # M1: H[t, j] = (Z_ft tile).T @ theta_k  -> node-major tiles
                # M2: S[d, n]@t = sum_i H[t,i].T @ V^T[i, :]   (FT out)
                # Emission is software-pipelined (M2 group g after M1 group
                # g+2) so the PSUM->SBUF drains hide under M2 matmuls.
                H = hpool.tile([128, T, NCH, C], fp8)
                hs = 1.0 if k == 0 else 0.25
                if not last:
                    Sout = spool.tile([128, TN], bf16)
                else:
                    Ofin = opool.tile([C, TN], bf16)

                def m1_group(tt):
                    pm1 = ps.tile([128, 1024], f32, tag="ps")
                    for u in range(2):
                        t = tt + u
                        for j in range(NCH):
                            nc.tensor.matmul(
                                pm1[:, u * 512 + j * 128: u * 512 + (j + 1) * 128],
                                Zin[:, t * N + j * 128: t * N + (j + 1) * 128],
                                th_sb[:, k, :], start=True, stop=True)
                    if (tt // 2) % 8 < CAST_DVE:
                        if hs == 1.0:
                            nc.vector.tensor_copy(H[:, tt:tt + 2, :, :], pm1[:])
                        else:
                            nc.vector.tensor_scalar(H[:, tt:tt + 2, :, :], pm1[:],
                                                    hs, None, OP.mult)
                    else:
                        if hs == 1.0:
                            nc.scalar.copy(H[:, tt:tt + 2, :, :], pm1[:])
                        else:
                            nc.scalar.mul(H[:, tt:tt + 2, :, :], pm1[:], hs)

                ds = 1.0 / (128.0 * hs)

                def m2_group(tt):
                    pm2 = ps.tile([128, 1024], f32, tag="ps")
                    for u in range(2):
                        t = tt + u
                        for p in range(0, NCH, 2):
                            nc.tensor.matmul(pm2[:, u * 512:(u + 1) * 512],
                                             H[:, t, p:p + 2, :],
                                             VT[:, k, p:p + 2, :],
                                             start=(p == 0), stop=(p == NCH - 2),
                                             perf_mode=DR)
                    lo, hi = tt * N, (tt + 2) * N
                    if not last:
                        if (tt // 2) % 8 < RELU_DVE:
                            nc.vector.tensor_scalar(Sout[:, lo:hi], pm2[:],
                                                    ds, 0.0, OP.mult, OP.max)
                        else:
                            nc.scalar.activation(Sout[:, lo:hi], pm2[:], AF.Relu,
                                                 scale=ds)
                    else:
                        sg = small.tile([128, 1024], bf16)
                        nc.scalar.activation(sg[:], pm2[:], AF.Sigmoid, scale=ds)
                        nc.vector.scalar_tensor_tensor(
                            Ofin[:, lo:hi], X[:, lo:hi], 0.0, sg[:],
                            OP.max, OP.add)

                # lag-3 interleave matched to the 3-slot PSUM ring: each M2
                # group (1.7us PE) paces the drain of the M1 tile 3 slots back
                m1_group(0)
                m1_group(2)
                m1_group(4)
                for g in range(0, T - 6, 2):
                    m2_group(g)
                    m1_group(g + 6)
                m2_group(T - 6)
                m2_group(T - 4)
                m2_group(T - 2)
                if not last:
                    Zin = Sout

            for h in range(8):
                nc.sync.dma_start(out_ext[b][:, h * (TN // 8):(h + 1) * (TN // 8)],
                                  Ofin[:, h * (TN // 8):(h + 1) * (TN // 8)])

    nc.compile()
    return nc


def _get_nc():
    if 'nc' not in _cache:
        _cache['nc'] = _build()
    return _cache['nc']


def _make_in_maps(x, adj, lambdas, thetas):
    bf16 = ml_dtypes.bfloat16
    x = np.asarray(x, dtype=np.float32)
    adj = np.asarray(adj, dtype=np.float32)
    lam = np.asarray(lambdas, dtype=np.float32).reshape(1, K)
    th = np.asarray(thetas, dtype=np.float32).astype(bf16)
    in_maps = []
    for c in range(NCORES):
        sl = slice(c * BL, (c + 1) * BL)
        # feature-major x: [BL, C, T*N]
        xc = np.ascontiguousarray(x[sl].transpose(0, 3, 1, 2)).reshape(BL, C, TN)
        # adj^T: [BL, m, n]
        atc = np.ascontiguousarray(adj[sl].transpose(0, 2, 1))
        in_maps.append({
            "x": xc.astype(bf16),
            "adjt": atc.astype(bf16),
            "lambdas": lam,
            "thetas": th,
        })
    return in_maps


def _run_device(in_maps):
    """Compile (cached) + execute on the 8 NeuronCores; returns per-core
    'out' arrays. Raises on device failure."""
    from concourse.bass_utils import run_bass_kernel_spmd

    nc = _get_nc()
    res = run_bass_kernel_spmd(nc, in_maps, core_ids=list(range(NCORES)))
    _cache['last_results'] = res
    return [np.asarray(res.results[c]["out"]) for c in range(NCORES)]


_BF16_KEYS = ("x", "adjt", "thetas")


def _subproc_main(in_path, out_path):
    bf16 = ml_dtypes.bfloat16
    data = np.load(in_path)
    in_maps = []
    for c in range(NCORES):
        m = {}
        for k in ("x", "adjt", "lambdas", "thetas"):
            v = data[f"{k}_{c}"]
            m[k] = v.view(bf16) if k in _BF16_KEYS else v
        in_maps.append(m)
    outs = _run_device(in_maps)
    np.savez(out_path, **{f"out_{c}": np.asarray(outs[c]).view(np.uint16)
                          for c in range(NCORES)})


def _run_device_subprocess(in_maps):
    """Fallback: run in a fresh interpreter. A wedged NeuronCore poisons the
    in-process PJRT client irrecoverably, but a new process recovers."""
    import os
    import subprocess
    import sys as _sys
    import tempfile

    d = tempfile.mkdtemp()
    in_path = os.path.join(d, "in.npz")
    out_path = os.path.join(d, "out.npz")
    arrs = {}
    for c, m in enumerate(in_maps):
        for k, v in m.items():
            arrs[f"{k}_{c}"] = v.view(np.uint16) if k in _BF16_KEYS else v
    np.savez(in_path, **arrs)
    mod_dir = os.path.dirname(os.path.abspath(__file__))
    code = (
        "import sys; sys.path.insert(0, %r); import kernel; "
        "kernel._subproc_main(%r, %r)" % (mod_dir, in_path, out_path)
    )
    subprocess.run([_sys.executable, "-c", code], check=True, timeout=900)
    data = np.load(out_path)
    return [data[f"out_{c}"].view(ml_dtypes.bfloat16) for c in range(NCORES)]


def kernel(x, adj, lambdas, thetas):
    import time

    in_maps = _make_in_maps(x, adj, lambdas, thetas)
    _cache['last_in_maps'] = in_maps
    outs = None
    try:
        outs = _run_device(in_maps)
    except Exception:
        # Device wedge (NRT unrecoverable) poisons this process's client;
        # retry in fresh subprocesses.
        for attempt in range(3):
            try:
                time.sleep(2.0 * (attempt + 1))
                outs = _run_device_subprocess(in_maps)
                break
            except Exception:
                if attempt == 2:
                    raise

    out = np.empty((B, T, N, C), dtype=np.float32)
    for c in range(NCORES):
        o = outs[c].astype(np.float32)  # [BL, C, TN]
        out[c * BL:(c + 1) * BL] = o.reshape(BL, C, T, N).transpose(0, 2, 3, 1)
    return out


# revision 38
# speedup vs baseline: 1.2404x; 1.2404x over previous
"""AKGCN Trainium2 kernel — data-parallel over batch across 8 NeuronCores.

Reference computation (per batch b):
    lam_k = 1 + relu(lambdas[k]);  a_k = (2*lam_k-2)/lam_k;  c_k = 2/lam_k
    layer k:  H = Z @ theta_k                       (feature matmul, C=128)
              V = (a_k*I + c_k*adj) row-normalized  (node matmul, N=512)
              Z' = relu(V @ H)          (vectorized over T)
    out = relu(x) + sigmoid(Z_final)

Device strategy (per core, 4 batches):
  - Activations alternate layouts per matmul so NO on-device transposes
    are needed:
      * feature-major FT: [feat 128 partitions, (t, n) = 8192 free]
      * node-major tiles: [node_local 128, (t, chunk, feat)]
    theta-matmul (bf16) consumes FT (activation tiles as stationary lhsT,
    theta moving) and emits node-major tiles; the adj-matmul consumes
    node-major (H tiles as lhsT, V^T moving 512-wide) and emits FT.
  - adj is supplied PRE-TRANSPOSED (A^T) and x PRE-TRANSPOSED to
    feature-major by the host glue below; both cast to bf16 on host.
  - Row normalization on device: rowsum via ones^T @ A^T matmul,
    rinv ~ 1/(a + c*rowsum); V^T = 128*c*rinv[n]*(A^T + (a/c)*I), built
    once per batch for both layers (step-0 broadcast APs cover the k dim).
  - The adj-matmul runs in fp8e4 DoubleRow perf mode (2 fp8 MACs/cell per
    cycle, K=256 per matmul): V^T is prescaled by 128 to stay in fp8
    normal range and layer-2 H by 1/4; the epilogue activation `scale`
    undoes both before relu/sigmoid. Verified rel err ~2e-3 (fro).
  - Output written feature-major bf16; host restores [T,N,C] f32.
  - M1/M2 emission is software-pipelined (lag-3, matched to the 3-slot
    PSUM ring) and PSUM drains are split across DVE/ScalarE so the
    TensorE never stalls (stalls also cool the HAM clock gate, halving
    the matmul clock).
"""
import sys

if '/opt/trn_rl_repo' not in sys.path:
    sys.path.insert(0, '/opt/trn_rl_repo')

import numpy as np
import ml_dtypes


def _bcast_mid(ap, count):
    # insert a step-0 middle dim: [P, F] -> [P, count, F]
    import concourse.bass as bass
    return bass.AP(ap.tensor, ap.offset,
                   [list(ap.ap[0]), [0, count], list(ap.ap[1])])

B, T, N, C, K = 32, 16, 512, 128, 2
NCORES = 8
BL = B // NCORES  # batches per core
TN = T * N        # 8192
NCH = N // 128    # 4 node chunks
CAST_DVE = 1      # of 8 M1-psum drains per layer, how many go to DVE (rest ScalarE)
RELU_DVE = 3      # of 8 M2 relu drains, how many go to DVE

_cache = {}


def _build():
    from contextlib import ExitStack
    import concourse.tile as tile
    import concourse.mybir as mybir
    from concourse import bacc

    dt = mybir.dt
    f32, bf16, i32 = dt.float32, dt.bfloat16, dt.int32
    fp8 = dt.float8e4
    DR = mybir.MatmulPerfMode.DoubleRow
    AF = mybir.ActivationFunctionType
    OP = mybir.AluOpType

    nc = bacc.Bacc(None, target_bir_lowering=False)
    x_ext = nc.declare_dram_parameter("x", [BL, C, TN], bf16, isOutput=False)
    at_ext = nc.declare_dram_parameter("adjt", [BL, N, N], bf16, isOutput=False)
    lam_ext = nc.declare_dram_parameter("lambdas", [1, K], f32, isOutput=False)
    th_ext = nc.declare_dram_parameter("thetas", [K, C, C], bf16, isOutput=False)
    out_ext = nc.declare_dram_parameter("out", [BL, C, TN], bf16, isOutput=True)

    with tile.TileContext(nc) as tc, ExitStack() as ctx:
        const = ctx.enter_context(tc.tile_pool(name="const", bufs=1))
        xpool = ctx.enter_context(tc.tile_pool(name="x", bufs=2))
        atpool = ctx.enter_context(tc.tile_pool(name="at", bufs=2))
        vtpool = ctx.enter_context(tc.tile_pool(name="vt", bufs=2))
        hpool = ctx.enter_context(tc.tile_pool(name="h", bufs=2))
        spool = ctx.enter_context(tc.tile_pool(name="s", bufs=2))
        opool = ctx.enter_context(tc.tile_pool(name="o", bufs=2))
        small = ctx.enter_context(tc.tile_pool(name="small", bufs=2))
        ps = ctx.enter_context(tc.tile_pool(name="ps", bufs=3, space="PSUM"))
        psr = ctx.enter_context(tc.tile_pool(name="psr", bufs=1, space="PSUM"))

        # ---- batch-0 input DMAs first: the rowsum matmul only needs A^T ----
        preloaded = {}

        def emit_at_dma(b):
            AT = atpool.tile([128, NCH, N], bf16)
            atr = at_ext[b].rearrange("(i p) n -> p i n", p=128)
            for i in range(NCH):
                nc.sync.dma_start(AT[:, i, :], atr[:, i, :])
            return AT

        def emit_x_dma(b):
            X = xpool.tile([C, TN], bf16)
            for h in range(4):
                nc.sync.dma_start(X[:, h * (TN // 4):(h + 1) * (TN // 4)],
                                  x_ext[b][:, h * (TN // 4):(h + 1) * (TN // 4)])
            return X

        def emit_in_dma(b):
            return emit_at_dma(b), emit_x_dma(b)

        AT0 = emit_at_dma(0)

        # ---- one-time constants ----
        th_sb = const.tile([C, K, C], bf16)
        nc.sync.dma_start(th_sb[:], th_ext[:].rearrange("k c d -> c k d"))
        lam_raw = const.tile([1, K], f32)
        nc.sync.dma_start(lam_raw[:], lam_ext[:])
        preloaded[0] = (AT0, emit_x_dma(0))
        ones_sb = const.tile([128, 1], bf16)
        nc.vector.memset(ones_sb[:], 1.0)

        # HAM pre-warm: keep TensorE busy while input DMAs land so the clock
        # gate opens (4/8 -> 8/8) before real matmuls start.
        warm_sb = const.tile([128, 512], bf16)
        nc.vector.memset(warm_sb[:], 0.0)
        wps = psr.tile([1, N], f32, tag="rps")
        for w in range(6):
            nc.tensor.matmul(wps[0:1, 0:N], ones_sb[:, 0:1], warm_sb[:],
                             start=(w == 0), stop=(w == 5))

        # lam = 1 + relu(lam_raw); c = 2/lam; a = 2 - c
        lam_sb = const.tile([1, K], f32)
        nc.vector.tensor_scalar(lam_sb[:], lam_raw[:], 0.0, 1.0, OP.max, OP.add)
        ilam_sb = const.tile([1, K], f32)
        nc.vector.reciprocal(ilam_sb[:], lam_sb[:])
        c_sb = const.tile([1, K], f32)
        nc.vector.tensor_scalar(c_sb[:], ilam_sb[:], 2.0, None, OP.mult)
        a_sb = const.tile([1, K], f32)
        nc.vector.tensor_scalar(a_sb[:], c_sb[:], -1.0, 2.0, OP.mult, OP.add)
        # per-partition copies for scalar_tensor_tensor operands,
        # pre-scaled by 128 (fp8 V^T prescale; undone in the epilogue scale)
        c_bc = const.tile([128, K], f32)
        nc.gpsimd.partition_broadcast(c_bc[:], c_sb[:])
        a_bc = const.tile([128, K], f32)
        nc.gpsimd.partition_broadcast(a_bc[:], a_sb[:])
        ci_bc = const.tile([128, K], f32)
        nc.vector.reciprocal(ci_bc[:], c_bc[:])
        ac_bc = const.tile([128, K], f32)
        nc.vector.tensor_tensor(ac_bc[:], a_bc[:], ci_bc[:], OP.mult)
        cs128_sb = const.tile([1, K], f32)
        nc.vector.tensor_scalar(cs128_sb[:], c_sb[:], 128.0, None, OP.mult)

        # eye128 (f32) via iota(f - p) == 0
        iota_sb = const.tile([128, 128], i32)
        nc.gpsimd.iota(iota_sb[:], pattern=[[1, 128]], base=0, channel_multiplier=-1)
        eye_sb = const.tile([128, 128], f32)
        nc.vector.tensor_scalar(eye_sb[:], iota_sb[:], 0, None, OP.is_equal)
        # eyeac[:, k, :] = (a_k/c_k) * I
        eyeac = const.tile([128, K, 128], f32)
        for k in range(K):
            nc.vector.tensor_scalar(eyeac[:, k, :], eye_sb[:],
                                    ac_bc[:, k:k + 1], None, OP.mult)

        def emit_norm(b):
            if b in preloaded:
                AT, X = preloaded.pop(b)
            else:
                AT, X = emit_in_dma(b)

            # rowsum[n] = sum_m A[n, m] = ones^T @ A^T   -> [1, N] in PSUM
            rps = psr.tile([1, N], f32, tag="rps")
            for i in range(NCH):
                nc.tensor.matmul(rps[0:1, 0:N], ones_sb[:, 0:1], AT[:, i, :],
                                 start=(i == 0), stop=(i == NCH - 1))
            # V^T_k = (128*c_k*rinv_k[n]) * (A^T + (a_k/c_k)*I)   (fp8)
            # RBC[:, k, n] = 128*c_k*rinv_k[n]; both k processed per op via
            # a step-0 broadcast of A^T along the k dim.
            VT = vtpool.tile([128, K, NCH, N], fp8)
            RBC = small.tile([128, K, N], f32)
            for k in range(K):
                rr = small.tile([1, N], f32)
                nc.vector.tensor_scalar(rr[:], rps[0:1, 0:N],
                                        c_sb[0:1, k:k + 1], a_sb[0:1, k:k + 1],
                                        OP.mult, OP.add)
                rinv = small.tile([1, N], f32)
                nc.vector.reciprocal_approx_fast(rinv[:], rr[:])
                rsc = small.tile([1, N], f32)
                nc.vector.tensor_scalar(rsc[:], rinv[:],
                                        cs128_sb[0:1, k:k + 1], None, OP.mult)
                nc.gpsimd.partition_broadcast(RBC[:, k, :], rsc[:])
            for i in range(NCH):
                nc.vector.tensor_tensor(VT[:, :, i, :], RBC[:],
                                        _bcast_mid(AT[:, i, :], K), OP.mult)
                dgs = small.tile([128, K, 128], f32)
                nc.vector.tensor_tensor(dgs[:], eyeac[:],
                                        _bcast_mid(AT[:, i, i * 128:(i + 1) * 128], K),
                                        OP.add)
                nc.vector.tensor_tensor(VT[:, :, i, i * 128:(i + 1) * 128],
                                        RBC[:, :, i * 128:(i + 1) * 128],
                                        dgs[:], OP.mult)
            return AT, VT, X

        for b in range(BL):
            AT, VT, X = emit_norm(b)
            Zin = X
            for k in range(K):
                last = (k == K - 1)
                # M1: H[t, j] = (Z_ft tile).T @ theta_k  -> node-major tiles
                # M2: S[d, n]@t = sum_i H[t,i].T @ V^T[i, :]   (FT out)
                # Emission is software-pipelined (M2 group g after M1 group
                # g+2) so the PSUM->SBUF drains hide under M2 matmuls.
                H = hpool.tile([128, T, NCH, C], fp8)
                hs = 1.0 if k == 0 else 0.25
                if not last:
                    Sout = spool.tile([128, TN], bf16)
                else:
                    Ofin = opool.tile([C, TN], bf16)

                def m1_group(tt):
                    pm1 = ps.tile([128, 1024], f32, tag="ps")
                    for u in range(2):
                        t = tt + u
                        for j in range(NCH):
                            nc.tensor.matmul(
                                pm1[:, u * 512 + j * 128: u * 512 + (j + 1) * 128],
                                Zin[:, t * N + j * 128: t * N + (j + 1) * 128],
                                th_sb[:, k, :], start=True, stop=True)
                    if (tt // 2) % 8 < CAST_DVE:
                        if hs == 1.0:
                            nc.vector.tensor_copy(H[:, tt:tt + 2, :, :], pm1[:])
                        else:
                            nc.vector.tensor_scalar(H[:, tt:tt + 2, :, :], pm1[:],
                                                    hs, None, OP.mult)
                    else:
                        if hs == 1.0:
                            nc.scalar.copy(H[:, tt:tt + 2, :, :], pm1[:])
                        else:
                            nc.scalar.mul(H[:, tt:tt + 2, :, :], pm1[:], hs)

                ds = 1.0 / (128.0 * hs)

                def m2_group(tt):
                    pm2 = ps.tile([128, 1024], f32, tag="ps")
                    for u in range(2):
                        t = tt + u
                        for p in range(0, NCH, 2):
                            nc.tensor.matmul(pm2[:, u * 512:(u + 1) * 512],
                                             H[:, t, p:p + 2, :],
                                             VT[:, k, p:p + 2, :],
                                             start=(p == 0), stop=(p == NCH - 2),
                                             perf_mode=DR)
                    lo, hi = tt * N, (tt + 2) * N
                    if not last:
                        if (tt // 2) % 8 < RELU_DVE:
                            nc.vector.tensor_scalar(Sout[:, lo:hi], pm2[:],
                                                    ds, 0.0, OP.mult, OP.max)
                        else:
                            nc.scalar.activation(Sout[:, lo:hi], pm2[:], AF.Relu,
                                                 scale=ds)
                    else:
                        sg = small.tile([128, 1024], bf16)
                        nc.scalar.activation(sg[:], pm2[:], AF.Sigmoid, scale=ds)
                        nc.vector.scalar_tensor_tensor(
                            Ofin[:, lo:hi], X[:, lo:hi], 0.0, sg[:],
                            OP.max, OP.add)

                # lag-3 interleave matched to the 3-slot PSUM ring: each M2
                # group (1.7us PE) paces the drain of the M1 tile 3 slots back
                m1_group(0)
                m1_group(2)
                m1_group(4)
                for g in range(0, T - 6, 2):
                    m2_group(g)
                    m1_group(g + 6)
                m2_group(T - 6)
                m2_group(T - 4)
                m2_group(T - 2)
                if not last:
                    Zin = Sout

            for h in range(8):
                nc.sync.dma_start(out_ext[b][:, h * (TN // 8):(h + 1) * (TN // 8)],
                                  Ofin[:, h * (TN // 8):(h + 1) * (TN // 8)])

    nc.compile()
    return nc


def _get_nc():
    if 'nc' not in _cache:
        _cache['nc'] = _build()
    return _cache['nc']


def _make_in_maps(x, adj, lambdas, thetas):
    bf16 = ml_dtypes.bfloat16
    x = np.asarray(x, dtype=np.float32)
    adj = np.asarray(adj, dtype=np.float32)
    lam = np.asarray(lambdas, dtype=np.float32).reshape(1, K)
    th = np.asarray(thetas, dtype=np.float32).astype(bf16)
    in_maps = []
    for c in range(NCORES):
        sl = slice(c * BL, (c + 1) * BL)
        # feature-major x: [BL, C, T*N]
        xc = np.ascontiguousarray(x[sl].transpose(0, 3, 1, 2)).reshape(BL, C, TN)
        # adj^T: [BL, m, n]
        atc = np.ascontiguousarray(adj[sl].transpose(0, 2, 1))
        in_maps.append({
            "x": xc.astype(bf16),
            "adjt": atc.astype(bf16),
            "lambdas": lam,
            "thetas": th,
        })
    return in_maps


def _run_device(in_maps):
    """Compile (cached) + execute on the 8 NeuronCores; returns per-core
    'out' arrays. Raises on device failure."""
    from concourse.bass_utils import run_bass_kernel_spmd

    nc = _get_nc()
    res = run_bass_kernel_spmd(nc, in_maps, core_ids=list(range(NCORES)))
    _cache['last_results'] = res
    return [np.asarray(res.results[c]["out"]) for c in range(NCORES)]


_BF16_KEYS = ("x", "adjt", "thetas")


def _subproc_main(in_path, out_path):
    bf16 = ml_dtypes.bfloat16
    data = np.load(in_path)
    in_maps = []
    for c in range(NCORES):
        m = {}
        for k in ("x", "adjt", "lambdas", "thetas"):
            v = data[f"{k}_{c}"]
            m[k] = v.view(bf16) if k in _BF16_KEYS else v
        in_maps.append(m)
    outs = _run_device(in_maps)
    np.savez(out_path, **{f"out_{c}": np.asarray(outs[c]).view(np.uint16)
                          for c in range(NCORES)})


def _run_device_subprocess(in_maps):
    """Fallback: run in a fresh interpreter. A wedged NeuronCore poisons the
    in-process PJRT client irrecoverably, but a new process recovers."""
    import os
    import subprocess
    import sys as _sys
    import tempfile

    d = tempfile.mkdtemp()
    in_path = os.path.join(d, "in.npz")
    out_path = os.path.join(d, "out.npz")
    arrs = {}
    for c, m in enumerate(in_maps):
        for k, v in m.items():
            arrs[f"{k}_{c}"] = v.view(np.uint16) if k in _BF16_KEYS else v
    np.savez(in_path, **arrs)
    mod_dir = os.path.dirname(os.path.abspath(__file__))
    code = (
        "import sys; sys.path.insert(0, %r); import kernel; "
        "kernel._subproc_main(%r, %r)" % (mod_dir, in_path, out_path)
    )
    subprocess.run([_sys.executable, "-c", code], check=True, timeout=900)
    data = np.load(out_path)
    return [data[f"out_{c}"].view(ml_dtypes.bfloat16) for c in range(NCORES)]


def kernel(x, adj, lambdas, thetas):
    import time

    in_maps = _make_in_maps(x, adj, lambdas, thetas)
    _cache['last_in_maps'] = in_maps
    outs = None
    try:
        outs = _run_device(in_maps)
    except Exception:
        # Device wedge (NRT unrecoverable) poisons this process's client;
        # retry in fresh subprocesses.
        for attempt in range(3):
            try:
                time.sleep(2.0 * (attempt + 1))
                outs = _run_device_subprocess(in_maps)
                break
            except Exception:
                if attempt == 2:
                    raise

    out = np.empty((B, T, N, C), dtype=np.float32)
    for c in range(NCORES):
        o = outs[c].astype(np.float32)  # [BL, C, TN]
        out[c * BL:(c + 1) * BL] = o.reshape(BL, C, T, N).transpose(0, 2, 3, 1)
    return out


# revision 39
# speedup vs baseline: 1.2494x; 1.0073x over previous
"""AKGCN Trainium2 kernel — data-parallel over batch across 8 NeuronCores.

Reference computation (per batch b):
    lam_k = 1 + relu(lambdas[k]);  a_k = (2*lam_k-2)/lam_k;  c_k = 2/lam_k
    layer k:  H = Z @ theta_k                       (feature matmul, C=128)
              V = (a_k*I + c_k*adj) row-normalized  (node matmul, N=512)
              Z' = relu(V @ H)          (vectorized over T)
    out = relu(x) + sigmoid(Z_final)

Device strategy (per core, 4 batches):
  - Activations alternate layouts per matmul so NO on-device transposes
    are needed:
      * feature-major FT: [feat 128 partitions, (t, n) = 8192 free]
      * node-major tiles: [node_local 128, (t, chunk, feat)]
    theta-matmul (bf16) consumes FT (activation tiles as stationary lhsT,
    theta moving) and emits node-major tiles; the adj-matmul consumes
    node-major (H tiles as lhsT, V^T moving 512-wide) and emits FT.
  - adj is supplied PRE-TRANSPOSED (A^T) and x PRE-TRANSPOSED to
    feature-major by the host glue below; both cast to bf16 on host.
  - Row normalization on device: rowsum via ones^T @ A^T matmul,
    rinv ~ 1/(a + c*rowsum); V^T = 128*c*rinv[n]*(A^T + (a/c)*I), built
    once per batch for both layers (step-0 broadcast APs cover the k dim).
  - The adj-matmul runs in fp8e4 DoubleRow perf mode (2 fp8 MACs/cell per
    cycle, K=256 per matmul): V^T is prescaled by 128 to stay in fp8
    normal range and layer-2 H by 1/4; the epilogue activation `scale`
    undoes both before relu/sigmoid. Verified rel err ~2e-3 (fro).
  - Output written feature-major bf16; host restores [T,N,C] f32.
  - M1/M2 emission is software-pipelined (lag-3, matched to the 3-slot
    PSUM ring) and PSUM drains are split across DVE/ScalarE so the
    TensorE never stalls (stalls also cool the HAM clock gate, halving
    the matmul clock).
"""
import sys

if '/opt/trn_rl_repo' not in sys.path:
    sys.path.insert(0, '/opt/trn_rl_repo')

import numpy as np
import ml_dtypes


def _bcast_mid(ap, count):
    # insert a step-0 middle dim: [P, F] -> [P, count, F]
    import concourse.bass as bass
    return bass.AP(ap.tensor, ap.offset,
                   [list(ap.ap[0]), [0, count], list(ap.ap[1])])

B, T, N, C, K = 32, 16, 512, 128, 2
NCORES = 8
BL = B // NCORES  # batches per core
TN = T * N        # 8192
NCH = N // 128    # 4 node chunks
CAST_DVE = 1      # of 8 M1-psum drains per layer, how many go to DVE (rest ScalarE)
RELU_DVE = 5      # of 8 M2 relu drains, how many go to DVE

_cache = {}


def _build():
    from contextlib import ExitStack
    import concourse.tile as tile
    import concourse.mybir as mybir
    from concourse import bacc

    dt = mybir.dt
    f32, bf16, i32 = dt.float32, dt.bfloat16, dt.int32
    fp8 = dt.float8e4
    DR = mybir.MatmulPerfMode.DoubleRow
    AF = mybir.ActivationFunctionType
    OP = mybir.AluOpType

    nc = bacc.Bacc(None, target_bir_lowering=False)
    x_ext = nc.declare_dram_parameter("x", [BL, C, TN], bf16, isOutput=False)
    at_ext = nc.declare_dram_parameter("adjt", [BL, N, N], bf16, isOutput=False)
    lam_ext = nc.declare_dram_parameter("lambdas", [1, K], f32, isOutput=False)
    th_ext = nc.declare_dram_parameter("thetas", [K, C, C], bf16, isOutput=False)
    out_ext = nc.declare_dram_parameter("out", [BL, C, TN], bf16, isOutput=True)

    with tile.TileContext(nc) as tc, ExitStack() as ctx:
        const = ctx.enter_context(tc.tile_pool(name="const", bufs=1))
        xpool = ctx.enter_context(tc.tile_pool(name="x", bufs=2))
        atpool = ctx.enter_context(tc.tile_pool(name="at", bufs=2))
        vtpool = ctx.enter_context(tc.tile_pool(name="vt", bufs=2))
        hpool = ctx.enter_context(tc.tile_pool(name="h", bufs=2))
        spool = ctx.enter_context(tc.tile_pool(name="s", bufs=2))
        opool = ctx.enter_context(tc.tile_pool(name="o", bufs=2))
        small = ctx.enter_context(tc.tile_pool(name="small", bufs=2))
        ps = ctx.enter_context(tc.tile_pool(name="ps", bufs=3, space="PSUM"))
        psr = ctx.enter_context(tc.tile_pool(name="psr", bufs=1, space="PSUM"))

        # ---- batch-0 input DMAs first: the rowsum matmul only needs A^T ----
        preloaded = {}

        def emit_at_dma(b):
            AT = atpool.tile([128, NCH, N], bf16)
            atr = at_ext[b].rearrange("(i p) n -> p i n", p=128)
            for i in range(NCH):
                nc.sync.dma_start(AT[:, i, :], atr[:, i, :])
            return AT

        def emit_x_dma(b):
            X = xpool.tile([C, TN], bf16)
            for h in range(4):
                nc.sync.dma_start(X[:, h * (TN // 4):(h + 1) * (TN // 4)],
                                  x_ext[b][:, h * (TN // 4):(h + 1) * (TN // 4)])
            return X

        def emit_in_dma(b):
            return emit_at_dma(b), emit_x_dma(b)

        AT0 = emit_at_dma(0)

        # ---- one-time constants ----
        th_sb = const.tile([C, K, C], bf16)
        nc.sync.dma_start(th_sb[:], th_ext[:].rearrange("k c d -> c k d"))
        lam_raw = const.tile([1, K], f32)
        nc.sync.dma_start(lam_raw[:], lam_ext[:])
        preloaded[0] = (AT0, emit_x_dma(0))
        ones_sb = const.tile([128, 1], bf16)
        nc.vector.memset(ones_sb[:], 1.0)

        # HAM pre-warm: keep TensorE busy while input DMAs land so the clock
        # gate opens (4/8 -> 8/8) before real matmuls start.
        warm_sb = const.tile([128, 512], bf16)
        nc.vector.memset(warm_sb[:], 0.0)
        wps = psr.tile([1, N], f32, tag="rps")
        for w in range(6):
            nc.tensor.matmul(wps[0:1, 0:N], ones_sb[:, 0:1], warm_sb[:],
                             start=(w == 0), stop=(w == 5))

        # lam = 1 + relu(lam_raw); c = 2/lam; a = 2 - c
        lam_sb = const.tile([1, K], f32)
        nc.vector.tensor_scalar(lam_sb[:], lam_raw[:], 0.0, 1.0, OP.max, OP.add)
        ilam_sb = const.tile([1, K], f32)
        nc.vector.reciprocal(ilam_sb[:], lam_sb[:])
        c_sb = const.tile([1, K], f32)
        nc.vector.tensor_scalar(c_sb[:], ilam_sb[:], 2.0, None, OP.mult)
        a_sb = const.tile([1, K], f32)
        nc.vector.tensor_scalar(a_sb[:], c_sb[:], -1.0, 2.0, OP.mult, OP.add)
        # per-partition copies for scalar_tensor_tensor operands,
        # pre-scaled by 128 (fp8 V^T prescale; undone in the epilogue scale)
        c_bc = const.tile([128, K], f32)
        nc.gpsimd.partition_broadcast(c_bc[:], c_sb[:])
        a_bc = const.tile([128, K], f32)
        nc.gpsimd.partition_broadcast(a_bc[:], a_sb[:])
        ci_bc = const.tile([128, K], f32)
        nc.vector.reciprocal(ci_bc[:], c_bc[:])
        ac_bc = const.tile([128, K], f32)
        nc.vector.tensor_tensor(ac_bc[:], a_bc[:], ci_bc[:], OP.mult)
        cs128_sb = const.tile([1, K], f32)
        nc.vector.tensor_scalar(cs128_sb[:], c_sb[:], 128.0, None, OP.mult)

        # eye128 (f32) via iota(f - p) == 0
        iota_sb = const.tile([128, 128], i32)
        nc.gpsimd.iota(iota_sb[:], pattern=[[1, 128]], base=0, channel_multiplier=-1)
        eye_sb = const.tile([128, 128], f32)
        nc.vector.tensor_scalar(eye_sb[:], iota_sb[:], 0, None, OP.is_equal)
        # eyeac[:, k, :] = (a_k/c_k) * I
        eyeac = const.tile([128, K, 128], f32)
        for k in range(K):
            nc.vector.tensor_scalar(eyeac[:, k, :], eye_sb[:],
                                    ac_bc[:, k:k + 1], None, OP.mult)

        def emit_norm(b):
            if b in preloaded:
                AT, X = preloaded.pop(b)
            else:
                AT, X = emit_in_dma(b)

            # rowsum[n] = sum_m A[n, m] = ones^T @ A^T   -> [1, N] in PSUM
            rps = psr.tile([1, N], f32, tag="rps")
            for i in range(NCH):
                nc.tensor.matmul(rps[0:1, 0:N], ones_sb[:, 0:1], AT[:, i, :],
                                 start=(i == 0), stop=(i == NCH - 1))
            # V^T_k = (128*c_k*rinv_k[n]) * (A^T + (a_k/c_k)*I)   (fp8)
            # RBC[:, k, n] = 128*c_k*rinv_k[n]; both k processed per op via
            # a step-0 broadcast of A^T along the k dim.
            VT = vtpool.tile([128, K, NCH, N], fp8)
            RBC = small.tile([128, K, N], f32)
            for k in range(K):
                rr = small.tile([1, N], f32)
                nc.vector.tensor_scalar(rr[:], rps[0:1, 0:N],
                                        c_sb[0:1, k:k + 1], a_sb[0:1, k:k + 1],
                                        OP.mult, OP.add)
                rinv = small.tile([1, N], f32)
                nc.vector.reciprocal_approx_fast(rinv[:], rr[:])
                rsc = small.tile([1, N], f32)
                nc.vector.tensor_scalar(rsc[:], rinv[:],
                                        cs128_sb[0:1, k:k + 1], None, OP.mult)
                nc.gpsimd.partition_broadcast(RBC[:, k, :], rsc[:])
            for i in range(NCH):
                nc.vector.tensor_tensor(VT[:, :, i, :], RBC[:],
                                        _bcast_mid(AT[:, i, :], K), OP.mult)
                dgs = small.tile([128, K, 128], f32)
                nc.vector.tensor_tensor(dgs[:], eyeac[:],
                                        _bcast_mid(AT[:, i, i * 128:(i + 1) * 128], K),
                                        OP.add)
                nc.vector.tensor_tensor(VT[:, :, i, i * 128:(i + 1) * 128],
                                        RBC[:, :, i * 128:(i + 1) * 128],
                                        dgs[:], OP.mult)
            return AT, VT, X

        for b in range(BL):
            AT, VT, X = emit_norm(b)
            Zin = X
            for k in range(K):
                last = (k == K - 1)
                # M1: H[t, j] = (Z_ft tile).T @ theta_k  -> node-major tiles
                # M2: S[d, n]@t = sum_i H[t,i].T @ V^T[i, :]   (FT out)
                # Emission is software-pipelined (M2 group g after M1 group
                # g+2) so the PSUM->SBUF drains hide under M2 matmuls.
                H = hpool.tile([128, T, NCH, C], fp8)
                hs = 1.0 if k == 0 else 0.25
                if not last:
                    Sout = spool.tile([128, TN], bf16)
                else:
                    Ofin = opool.tile([C, TN], bf16)

                def m1_group(tt):
                    pm1 = ps.tile([128, 1024], f32, tag="ps")
                    for u in range(2):
                        t = tt + u
                        for j in range(NCH):
                            nc.tensor.matmul(
                                pm1[:, u * 512 + j * 128: u * 512 + (j + 1) * 128],
                                Zin[:, t * N + j * 128: t * N + (j + 1) * 128],
                                th_sb[:, k, :], start=True, stop=True)
                    if (tt // 2) % 8 < CAST_DVE:
                        if hs == 1.0:
                            nc.vector.tensor_copy(H[:, tt:tt + 2, :, :], pm1[:])
                        else:
                            nc.vector.tensor_scalar(H[:, tt:tt + 2, :, :], pm1[:],
                                                    hs, None, OP.mult)
                    else:
                        if hs == 1.0:
                            nc.scalar.copy(H[:, tt:tt + 2, :, :], pm1[:])
                        else:
                            nc.scalar.mul(H[:, tt:tt + 2, :, :], pm1[:], hs)

                ds = 1.0 / (128.0 * hs)

                def m2_group(tt):
                    pm2 = ps.tile([128, 1024], f32, tag="ps")
                    for u in range(2):
                        t = tt + u
                        for p in range(0, NCH, 2):
                            nc.tensor.matmul(pm2[:, u * 512:(u + 1) * 512],
                                             H[:, t, p:p + 2, :],
                                             VT[:, k, p:p + 2, :],
                                             start=(p == 0), stop=(p == NCH - 2),
                                             perf_mode=DR)
                    lo, hi = tt * N, (tt + 2) * N
                    if not last:
                        if (tt // 2) % 8 < RELU_DVE:
                            nc.vector.tensor_scalar(Sout[:, lo:hi], pm2[:],
                                                    ds, 0.0, OP.mult, OP.max)
                        else:
                            nc.scalar.activation(Sout[:, lo:hi], pm2[:], AF.Relu,
                                                 scale=ds)
                    else:
                        sg = small.tile([128, 1024], bf16)
                        nc.scalar.activation(sg[:], pm2[:], AF.Sigmoid, scale=ds)
                        nc.vector.scalar_tensor_tensor(
                            Ofin[:, lo:hi], X[:, lo:hi], 0.0, sg[:],
                            OP.max, OP.add)

                # lag-3 interleave matched to the 3-slot PSUM ring: each M2
                # group (1.7us PE) paces the drain of the M1 tile 3 slots back
                m1_group(0)
                m1_group(2)
                m1_group(4)
                for g in range(0, T - 6, 2):
                    m2_group(g)
                    m1_group(g + 6)
                m2_group(T - 6)
                m2_group(T - 4)
                m2_group(T - 2)
                if not last:
                    Zin = Sout

            for h in range(8):
                nc.sync.dma_start(out_ext[b][:, h * (TN // 8):(h + 1) * (TN // 8)],
                                  Ofin[:, h * (TN // 8):(h + 1) * (TN // 8)])

    nc.compile()
    return nc


def _get_nc():
    if 'nc' not in _cache:
        _cache['nc'] = _build()
    return _cache['nc']


def _make_in_maps(x, adj, lambdas, thetas):
    bf16 = ml_dtypes.bfloat16
    x = np.asarray(x, dtype=np.float32)
    adj = np.asarray(adj, dtype=np.float32)
    lam = np.asarray(lambdas, dtype=np.float32).reshape(1, K)
    th = np.asarray(thetas, dtype=np.float32).astype(bf16)
    in_maps = []
    for c in range(NCORES):
        sl = slice(c * BL, (c + 1) * BL)
        # feature-major x: [BL, C, T*N]
        xc = np.ascontiguousarray(x[sl].transpose(0, 3, 1, 2)).reshape(BL, C, TN)
        # adj^T: [BL, m, n]
        atc = np.ascontiguousarray(adj[sl].transpose(0, 2, 1))
        in_maps.append({
            "x": xc.astype(bf16),
            "adjt": atc.astype(bf16),
            "lambdas": lam,
            "thetas": th,
        })
    return in_maps


def _run_device(in_maps):
    """Compile (cached) + execute on the 8 NeuronCores; returns per-core
    'out' arrays. Raises on device failure."""
    from concourse.bass_utils import run_bass_kernel_spmd

    nc = _get_nc()
    res = run_bass_kernel_spmd(nc, in_maps, core_ids=list(range(NCORES)))
    _cache['last_results'] = res
    return [np.asarray(res.results[c]["out"]) for c in range(NCORES)]


_BF16_KEYS = ("x", "adjt", "thetas")


def _subproc_main(in_path, out_path):
    bf16 = ml_dtypes.bfloat16
    data = np.load(in_path)
    in_maps = []
    for c in range(NCORES):
        m = {}
        for k in ("x", "adjt", "lambdas", "thetas"):
            v = data[f"{k}_{c}"]
            m[k] = v.view(bf16) if k in _BF16_KEYS else v
        in_maps.append(m)
    outs = _run_device(in_maps)
    np.savez(out_path, **{f"out_{c}": np.asarray(outs[c]).view(np.uint16)
                          for c in range(NCORES)})


def _run_device_subprocess(in_maps):
    """Fallback: run in a fresh interpreter. A wedged NeuronCore poisons the
    in-process PJRT client irrecoverably, but a new process recovers."""
    import os
    import subprocess
    import sys as _sys
    import tempfile

    d = tempfile.mkdtemp()
    in_path = os.path.join(d, "in.npz")
    out_path = os.path.join(d, "out.npz")
    arrs = {}
    for c, m in enumerate(in_maps):
        for k, v in m.items():
            arrs[f"{k}_{c}"] = v.view(np.uint16) if k in _BF16_KEYS else v
    np.savez(in_path, **arrs)
    mod_dir = os.path.dirname(os.path.abspath(__file__))
    code = (
        "import sys; sys.path.insert(0, %r); import kernel; "
        "kernel._subproc_main(%r, %r)" % (mod_dir, in_path, out_path)
    )
    subprocess.run([_sys.executable, "-c", code], check=True, timeout=900)
    data = np.load(out_path)
    return [data[f"out_{c}"].view(ml_dtypes.bfloat16) for c in range(NCORES)]


def kernel(x, adj, lambdas, thetas):
    import time

    in_maps = _make_in_maps(x, adj, lambdas, thetas)
    _cache['last_in_maps'] = in_maps
    outs = None
    try:
        outs = _run_device(in_maps)
    except Exception:
        # Device wedge (NRT unrecoverable) poisons this process's client;
        # retry in fresh subprocesses.
        for attempt in range(3):
            try:
                time.sleep(2.0 * (attempt + 1))
                outs = _run_device_subprocess(in_maps)
                break
            except Exception:
                if attempt == 2:
                    raise

    out = np.empty((B, T, N, C), dtype=np.float32)
    for c in range(NCORES):
        o = outs[c].astype(np.float32)  # [BL, C, TN]
        out[c * BL:(c + 1) * BL] = o.reshape(BL, C, T, N).transpose(0, 2, 3, 1)
    return out
